# revision 21
# baseline (speedup 1.0000x reference)
"""Trainium2 Bass kernel for multi-head attention with RoPE (nn_Attention).

Reference computation (B=1, N=2048, D=1024, 16 heads, hd=64):
    q = x @ wq.T; k = x @ wk.T; v = x @ wv.T      (reshaped to heads)
    q, k = rope(q), rope(k)
    out = softmax(q k^T / sqrt(hd)) v              (non-causal, full)
    return (out reshaped) @ wp.T

Sharding: tensor-parallel over heads — each of the 8 cores owns 2 heads for
QKV projection + SDPA, then an AllToAll redistributes the attention output
so each core computes the final projection for its 256 sequence rows with
the full wp.

Measured HW facts this kernel is shaped around (For_i slope probes):
  * the PE pays ~400ns whenever consecutive matmuls change tile geometry
    (contract/out partition config), so EVERY matmul here runs at the same
    (128,128) config: S stationaries are zero-padded per (k-tile, head) to
    128 contraction rows (the moving full-partition Qp read contributes 0
    from the other head's rows), and the softmax-normalization broadcast
    matmul is padded the same way (GEOM);
  * bf16 operand pairs run ~45ns/matmul faster than f32r (half-width
    LDWEIGHTS) and need no on-chip widening, so all matmul operands are
    bf16, straight from DRAM (BF16M); accumulation stays f32 in PSUM
    (rel err ~6e-3 vs the 2e-2 budget);
  * ACT exp costs ~(N+30)/1.2GHz with PSUM source — it is the pacing
    engine in steady state, so the PE stream is kept dense: deferred
    projection work is injected one ~1-ktile piece per k-tile slot
    (INJ_FINE) and each chunk's normalization tail is split into three
    pieces (reciprocal / broadcast / payload-mul+DMA) spread over the next
    chunk's slots and carried across the attention-call boundary
    (TAIL_SPREAD), so the in-order PE never blocks on a fresh DVE result.

The timing build software-pipelines iterations: the hardware loop body
holds TWO copies of the whole projection+attention stage with ping-pong
Q/K/V tiles and shared PSUM pools, so one copy's projections (PE/DVE/DMA)
overlap the other copy's softmax-bound attention.

Self-contained: only imports numpy + the concourse stack available in the
execution environment. kernel(**inputs) takes the full unsharded inputs and
returns the full output.
"""
import numpy as np

DIM = 1024
NHEADS = 16
HD = 64
SEQ = 2048
NCORES = 8
ROPE_BASE = 10000.0
HPC = NHEADS // NCORES      # heads per core = 2
CH = HPC * HD               # channels per core = 128
QCH = 512                   # q-chunk (free dim of S/P tiles)
NQC = SEQ // QCH            # 4
NKT = SEQ // 128            # 16 k-tiles
DCH = DIM // 128            # 8 contraction chunks

_CACHE = {}
_PARTS_MODE = "ab"
SPLIT_S = 0      # diag: split each S matmul into 2 halves
XT_POOL = 0     # widen xt on the Pool engine instead of DVE
INJ_FINE = 1    # flat, k-tile-granular injection of deferred projections
INJ_EARLY = 0   # emit injected piece before O(kt-1), widening the exp shadow
TAIL_SPREAD = 1  # spread emit_tail into per-kt pieces in the next chunk
GEOM = 1        # uniform (128,128) PE tile geometry: zero-padded S/rb stationaries
BF16M = 1       # bf16 matmul operands everywhere (halves LDWEIGHTS, no widening)
EXP_SPLIT = 0   # one exp per (kt, head): halves the exp->O wait granularity
PB = 0          # deeper SBUF pools (p_sb/qs/qct WAR slack; room freed by BF16M)

# Optional ACT/DVE softmax-exp split (0 = all exp on the scalar engine).
# Measured neutral at small widths — the two custom-DVE instructions cost
# ~650ns/k-tile fixed, which eats the ACT savings — so disabled.
DVE_W = 0
EXP_C = (1.0000677, 0.50037957, 0.1594775)


def _np_bf16():
    import concourse.mybir as mybir
    return mybir.dt.np(mybir.dt.bfloat16)


def _register_exp_ops():
    import re
    import concourse.dve_ops as dops
    from concourse.dve_spec import Spec, Src0, One, C0, C1, C2

    if "EXP_POLY_ANT" in dops.CUSTOM_DVE_SPECS:
        by = {o.name: o for o in dops.OPS}
        return by["EXP_POLY_ANT"], by["EXP_SQ8_ANT"]

    def np_poly(in0, in1, c0, c1, c2):
        f = np.float32
        t = np.asarray(in0, f)
        t2 = (t * t).astype(f)
        m1 = (t * np.asarray(c0, f)).astype(f)
        u1 = (m1 + f(1.0)).astype(f)
        m2 = (t * f(c2)).astype(f)
        u2 = (m2 + np.asarray(c1, f)).astype(f)
        return (u1 + (t2 * u2).astype(f)).astype(f)

    def np_sq8(in0, in1, c0, c1, c2):
        yv = np.asarray(in0, np.float32)
        for _ in range(8):
            yv = (yv * yv).astype(np.float32)
        return yv

    t2 = Src0 * Src0
    body1 = (Src0 * C0 + One) + t2 * (Src0 * C2 + C1)
    yb = Src0
    for _ in range(8):
        yb = yb * yb
    ops = []
    for name, body, ref in (("EXP_POLY_ANT", body1, np_poly),
                            ("EXP_SQ8_ANT", yb, np_sq8)):
        op = dops.DveOp(name, Spec(body=body, reference=ref), subdim=False,
                        uops_sha={})
        dops.OPS.append(op)
        dops.CUSTOM_DVE_SPECS[name] = op.spec
        dops._SUB_OPCODE_FOR_NAME[name] = (
            max(dops._SUB_OPCODE_FOR_NAME.values()) + 1)
        for ver in ("v3",):
            try:
                op.compile(ver)
            except ValueError as e:
                m = re.search(r'="([0-9a-f]+)"', str(e))
                assert m, f"no sha in error: {e}"
                op.uops_sha[ver] = m.group(1)
                op.compile(ver)
        ops.append(op)
    return ops


def _rope_tables():
    inv = 1.0 / (ROPE_BASE ** (np.arange(0, HD, 2, dtype=np.float64) / HD))
    t = np.arange(SEQ, dtype=np.float64)
    freqs = np.outer(t, inv)                      # [SEQ, 32]
    emb = np.concatenate([freqs, freqs], 1)       # [SEQ, 64]
    cosT = np.cos(emb).T                          # [64, SEQ]
    sinT = np.sin(emb).T
    sig = (np.arange(HD) + 32) % HD
    sT = sinT[sig]                                # shifted sin
    cos2 = np.concatenate([cosT, cosT], 0)        # [128, SEQ] (2 heads)
    s2 = np.concatenate([sT, sT], 0)
    return cos2, s2


def _r2t():
    # rotate-half matrix R (per head), block-diagonal over the 2 heads; we
    # pass R2.T as the stationary matmul operand.
    R = np.zeros((HD, HD), np.float64)
    for j in range(32):
        R[j, j + 32] = -1.0
        R[j + 32, j] = 1.0
    R2 = np.zeros((CH, CH), np.float64)
    R2[0:HD, 0:HD] = R
    R2[HD:CH, HD:CH] = R
    return np.ascontiguousarray(R2.T)


def _build(nrep=1, n_cores=NCORES, with_c=True, parts="ab"):
    global _PARTS_MODE
    _PARTS_MODE = parts
    import concourse.mybir as mybir
    import concourse.tile as tile
    from concourse import bacc
    from concourse.masks import make_identity

    F32 = mybir.dt.float32
    F32R = mybir.dt.float32r
    BF16 = mybir.dt.bfloat16
    MDT = BF16 if BF16M else F32R
    EXP = mybir.ActivationFunctionType.Exp

    if DVE_W:
        exp_op1, exp_op2 = _register_exp_ops()

    nc = bacc.Bacc("TRN2", target_bir_lowering=False, debug=False,
                   num_devices=n_cores)

    xt_ext = nc.dram_tensor("xt", [DIM, SEQ], BF16, kind="ExternalInput")
    wq_ext = nc.dram_tensor("wq_t", [DIM, CH], BF16, kind="ExternalInput")
    wk_ext = nc.dram_tensor("wk_t", [DIM, CH], BF16, kind="ExternalInput")
    wv_ext = nc.dram_tensor("wv_t", [DIM, CH], BF16, kind="ExternalInput")
    wp_ext = nc.dram_tensor("wp_t", [DIM, DIM], BF16, kind="ExternalInput")
    ck_ext = nc.dram_tensor("cos_k", [CH, SEQ], BF16, kind="ExternalInput")
    sk_ext = nc.dram_tensor("sin_k", [CH, SEQ], BF16, kind="ExternalInput")
    r2t_ext = nc.dram_tensor("r2t", [CH, CH], BF16, kind="ExternalInput")
    out_ext = nc.dram_tensor("out", [SEQ // NCORES, DIM], F32,
                             kind="ExternalOutput")
    a2a_in = nc.dram_tensor("a2a_in", [NCORES, CH, SEQ // NCORES], BF16)
    a2a_out = nc.dram_tensor("a2a_out", [NCORES, CH, SEQ // NCORES], BF16)

    with tile.TileContext(nc) as tc:
        with tc.tile_pool(name="persist", bufs=1) as P1:

            def make_set(sfx):
                Qp = P1.tile([CH, SEQ], MDT, tag="Qp" + sfx)
                if GEOM:
                    # per-(kt, head) zero-padded stationaries: the head's 64
                    # rows live at their natural partitions, the other 64
                    # rows stay zero so the full-partition moving Qp read
                    # contributes nothing from the other head. Keeps every
                    # matmul at the same (128,128) PE tile config - geometry
                    # switches cost ~400ns each on HW.
                    Kp = P1.tile([128, NKT, HPC, 128], MDT, tag="Kp" + sfx)
                    if BF16M:
                        nc.vector.memset(Kp[:], 0.0)
                    else:
                        nc.vector.memset(Kp[:].bitcast(F32), 0.0)
                else:
                    Kp = P1.tile([CH, SEQ], MDT, tag="Kp" + sfx)
                Vsb = P1.tile([128, NKT, HPC, HD + 1], MDT, tag="Vsb" + sfx)
                onescol = P1.tile([128, HD], F32R, tag="ones" + sfx)
                return (Qp, Kp, Vsb, onescol)

            shared = {}

            def run(nrep_eff):
                ident0 = P1.tile([128, 128], F32, name="ident0", tag="ident0")
                identr0 = P1.tile([128, 128], MDT, name="identr0",
                                  tag="identr0")
                make_identity(nc, ident0[:])
                nc.vector.tensor_copy(identr0[:], ident0[:])
                shared["identr"] = identr0
                if GEOM:
                    bpad = P1.tile([HD + 1, 128], F32R, name="bpad",
                                   tag="bpad")
                    nc.vector.memset(bpad[:].bitcast(F32), 0.0)
                    nc.vector.memset(bpad[HD:HD + 1, :].bitcast(F32), 1.0)
                    shared["bpad"] = bpad
                with (
                    tc.tile_pool(name="stA", bufs=1) as A_sb,
                    tc.tile_pool(name="stA2", bufs=3 if PB else 2) as A_db,
                    tc.tile_pool(name="psS", bufs=2, space="PSUM") as psS,
                    tc.tile_pool(name="psAcc", bufs=2, space="PSUM") as psAcc,
                    tc.tile_pool(name="psO", bufs=2, space="PSUM") as psO,
                    tc.tile_pool(name="stB", bufs=5 if PB else 3) as B_db,
                    tc.tile_pool(name="stBs", bufs=4 if PB else 3) as B_sm,
                    tc.tile_pool(name="stXf", bufs=3 if GEOM else 4) as Xf_db,
                ):
                    pools = (A_sb, A_db, psS, psAcc, psO, B_db, B_sm, Xf_db)
                    setA = make_set("A")
                    if nrep_eff == 1:
                        stage_proj(pools, *setA)
                        _attention(pools, *setA, None, None)
                    else:
                        assert nrep_eff % 2 == 0
                        setB = make_set("B")
                        # prologue: project set A once; each loop body then
                        # re-projects the OTHER set inside this set's
                        # attention (chunk-granular injection keeps the PE
                        # stream dense while ACT paces the softmax)
                        stage_proj(pools, *setA)
                        with tc.For_i(0, nrep_eff // 2, 1) as _i:
                            injB = stage_proj(pools, *setB, defer=True)
                            carry = _attention(pools, *setA, injB, None)
                            injA = stage_proj(pools, *setA, defer=True)
                            _attention(pools, *setB, injA, carry)

            def stage_proj(pools, Qp, Kp, Vsb, onescol, defer=False):
                (A_sb, A_db, psS, psAcc, psO, B_db, B_sm, Xf_db) = pools

                def init_ones():
                    aux1 = A_sb.tile([128, HD], F32, tag="aux1")
                    nc.vector.memset(aux1[:], 1.0)
                    nc.vector.tensor_copy(onescol[:], aux1[:])

                    nc.vector.tensor_copy(
                        Vsb[:, :, :, HD],
                        aux1[:, 0:NKT * HPC].rearrange("p (k h) -> p k h",
                                                       h=HPC))

                # ---- inputs: bf16 from DRAM; widened to f32r on chip.
                # All DMAs are emitted immediately (the SP queue prefetches
                # as staging slots free up); the widening copies and the
                # projections are returned as per-qc closures when defer=True
                # so they ride inside the other pipeline copy's attention.
                wq = A_sb.tile([128, DCH, CH], MDT, tag="wq")
                wk = A_sb.tile([128, DCH, CH], MDT, tag="wk")
                wv = A_sb.tile([128, DCH, CH], MDT, tag="wv")
                xt = A_sb.tile([128, DCH, SEQ], MDT, tag="xt")
                xt_r = xt_ext.rearrange("(c p) n -> p c n", p=128)
                ck = A_sb.tile([CH, SEQ], BF16, tag="ck")
                sk = A_sb.tile([CH, SEQ], BF16, tag="sk")
                if BF16M:
                    # bf16 operands feed the PE directly from DRAM: no
                    # staging, no widening copies
                    r2t = A_db.tile([CH, CH], BF16, tag="r2tf")
                    nc.sync.dma_start(
                        out=wk[:],
                        in_=wk_ext.rearrange("(c p) j -> p c j", p=128))
                    nc.sync.dma_start(out=r2t[:], in_=r2t_ext[:])
                    nc.sync.dma_start(
                        out=wv[:],
                        in_=wv_ext.rearrange("(c p) j -> p c j", p=128))
                    for d in range(DCH):
                        nc.sync.dma_start(out=xt[:, d, :], in_=xt_r[:, d, :])
                    nc.sync.dma_start(out=sk[:], in_=sk_ext[:])
                    nc.sync.dma_start(out=ck[:], in_=ck_ext[:])
                    nc.sync.dma_start(
                        out=wq[:],
                        in_=wq_ext.rearrange("(c p) j -> p c j", p=128))
                else:
                    wkf = A_db.tile([128, DCH, CH], BF16, tag="wf")
                    nc.sync.dma_start(
                        out=wkf[:],
                        in_=wk_ext.rearrange("(c p) j -> p c j", p=128))
                    r2t = A_sb.tile([CH, CH], F32R, tag="r2t")
                    r2tf = A_db.tile([CH, CH], BF16, tag="r2tf")
                    nc.sync.dma_start(out=r2tf[:], in_=r2t_ext[:])
                    wvf = A_db.tile([128, DCH, CH], BF16, tag="wf")
                    nc.sync.dma_start(
                        out=wvf[:],
                        in_=wv_ext.rearrange("(c p) j -> p c j", p=128))
                    xtfs = []
                    for d in range(DCH):
                        xtf = Xf_db.tile([128, SEQ], BF16, tag="xtf")
                        nc.sync.dma_start(out=xtf[:], in_=xt_r[:, d, :])
                        xtfs.append(xtf)
                    nc.sync.dma_start(out=sk[:], in_=sk_ext[:])
                    nc.sync.dma_start(out=ck[:], in_=ck_ext[:])
                    wqf = A_db.tile([128, DCH, CH], BF16, tag="wf")
                    nc.sync.dma_start(
                        out=wqf[:],
                        in_=wq_ext.rearrange("(c p) j -> p c j", p=128))

                identr = shared["identr"]

                def copies():
                    if not BF16M:
                        # weights widen on the idle gpsimd; xt on the DVE
                        # where the bf16->f32r copy runs in 2x mode
                        nc.gpsimd.tensor_copy(wk[:], wkf[:])
                        nc.vector.tensor_copy(r2t[:], r2tf[:])
                        nc.gpsimd.tensor_copy(wv[:], wvf[:])
                        xteng = nc.gpsimd if XT_POOL else nc.vector
                        for d in range(DCH):
                            xteng.tensor_copy(xt[:, d, :], xtfs[d])
                        nc.gpsimd.tensor_copy(wq[:], wqf[:])

                def rope_add(dst, qct, acc, c, is_k):
                    slc = slice(c * QCH, (c + 1) * QCH)
                    if not (GEOM and is_k):
                        nc.vector.tensor_add(dst[:, slc], qct[:], acc[:])
                        return
                    kts = slice(c * (QCH // 128), (c + 1) * (QCH // 128))
                    for h in range(HPC):
                        hp = slice(h * HD, (h + 1) * HD)
                        nc.vector.tensor_add(
                            dst[hp, kts, h, :],
                            qct[hp, :].rearrange("p (k n) -> p k n", n=128),
                            acc[hp, :].rearrange("p (k n) -> p k n", n=128))

                def proj_rope(w_sb, dst, c, is_k=False):
                    # shared K/Q chunk: project into a 1-bank accumulator,
                    # RoPE via two DVE muls + in-place rot matmul + add
                    slc = slice(c * QCH, (c + 1) * QCH)
                    acc = psAcc.tile([CH, QCH], F32, tag="acc")
                    for d in range(DCH):
                        nc.tensor.matmul(acc[:], w_sb[:, d, :], xt[:, d, slc],
                                         start=(d == 0), stop=(d == DCH - 1))
                    qs = A_db.tile([CH, QCH], MDT, tag="qs")
                    nc.vector.tensor_mul(qs[:], acc[:], sk[:, slc])
                    qct = A_db.tile([CH, QCH], MDT, tag="qct")
                    nc.vector.tensor_mul(qct[:], acc[:], ck[:, slc])
                    nc.tensor.matmul(acc[:], r2t[:], qs[:],
                                     start=True, stop=True)
                    rope_add(dst, qct, acc, c, is_k)

                def proj_K(c):
                    proj_rope(wk, Kp, c, is_k=True)

                def proj_Q(c):
                    proj_rope(wq, Qp, c)

                def proj_V(c):
                    # project, copy out of psum, PE-transpose into Vsb
                    slc = slice(c * QCH, (c + 1) * QCH)
                    vacc = psAcc.tile([CH, QCH], F32, tag="acc")
                    for d in range(DCH):
                        nc.tensor.matmul(vacc[:], wv[:, d, :], xt[:, d, slc],
                                         start=(d == 0), stop=(d == DCH - 1))
                    vt = A_db.tile([CH, QCH], MDT, tag="vt")
                    nc.vector.tensor_copy(vt[:], vacc[:])
                    for b in range(QCH // 128):
                        kti = c * (QCH // 128) + b
                        ps_t = psAcc.tile([128, 128], MDT, tag="acc")
                        nc.tensor.transpose(
                            ps_t[:], vt[:, b * 128:(b + 1) * 128], identr[:])
                        nc.vector.tensor_copy(
                            Vsb[:, kti, :, 0:HD],
                            ps_t[:].rearrange("p (h j) -> p h j", h=HPC))

                def proj_rope_pieces(w_sb, dst, c, is_k=False):
                    # same work as proj_rope, split into ~1-ktile PE pieces
                    slc = slice(c * QCH, (c + 1) * QCH)
                    st = {}

                    def p1():
                        st["acc"] = psAcc.tile([CH, QCH], F32, name="acc",
                                               tag="acc")
                        for d in range(DCH // 2):
                            nc.tensor.matmul(st["acc"][:], w_sb[:, d, :],
                                             xt[:, d, slc],
                                             start=(d == 0), stop=False)

                    def p2():
                        for d in range(DCH // 2, DCH):
                            nc.tensor.matmul(st["acc"][:], w_sb[:, d, :],
                                             xt[:, d, slc],
                                             start=False, stop=(d == DCH - 1))

                    def p3():
                        st["qs"] = A_db.tile([CH, QCH], MDT, name="qs",
                                             tag="qs")
                        nc.vector.tensor_mul(st["qs"][:], st["acc"][:],
                                             sk[:, slc])
                        st["qct"] = A_db.tile([CH, QCH], MDT, name="qct",
                                              tag="qct")
                        nc.vector.tensor_mul(st["qct"][:], st["acc"][:],
                                             ck[:, slc])
                        nc.tensor.matmul(st["acc"][:], r2t[:], st["qs"][:],
                                         start=True, stop=True)

                    def p4():
                        rope_add(dst, st["qct"], st["acc"], c, is_k)

                    return [p1, p2, p3, p4]

                def proj_V_pieces(c):
                    slc = slice(c * QCH, (c + 1) * QCH)
                    st = {}

                    def p1():
                        st["vacc"] = psAcc.tile([CH, QCH], F32, name="vacc",
                                                tag="acc")
                        for d in range(DCH // 2):
                            nc.tensor.matmul(st["vacc"][:], wv[:, d, :],
                                             xt[:, d, slc],
                                             start=(d == 0), stop=False)

                    def p2():
                        for d in range(DCH // 2, DCH):
                            nc.tensor.matmul(st["vacc"][:], wv[:, d, :],
                                             xt[:, d, slc],
                                             start=False, stop=(d == DCH - 1))

                    def trans(b):
                        kti = c * (QCH // 128) + b
                        ps_t = psAcc.tile([128, 128], MDT, name="ps_t",
                                          tag="acc")
                        nc.tensor.transpose(
                            ps_t[:], st["vt"][:, b * 128:(b + 1) * 128],
                            identr[:])
                        nc.vector.tensor_copy(
                            Vsb[:, kti, :, 0:HD],
                            ps_t[:].rearrange("p (h j) -> p h j", h=HPC))

                    def p3():
                        st["vt"] = A_db.tile([CH, QCH], MDT, name="vt",
                                             tag="vt")
                        nc.vector.tensor_copy(st["vt"][:], st["vacc"][:])
                        trans(0)
                        trans(1)

                    def p4():
                        trans(2)
                        trans(3)

                    return [p1, p2, p3, p4]

                if not defer:
                    copies()
                    init_ones()
                    for c in range(NQC):
                        proj_K(c)
                    for c in range(NQC):
                        proj_Q(c)
                    for c in range(NQC):
                        proj_V(c)
                    return None
                if INJ_FINE:
                    pieces = [copies, init_ones]
                    for c in range(NQC):
                        pieces += proj_rope_pieces(wk, Kp, c, is_k=True)
                    for c in range(NQC):
                        pieces += proj_rope_pieces(wq, Qp, c)
                    for c in range(NQC):
                        pieces += proj_V_pieces(c)
                    return pieces
                # four projection chunks per qc slot (~7.8us PE each)
                # so no attention chunk's PE slack is oversubscribed
                return {
                    0: [copies, init_ones],
                    1: [lambda: proj_K(0), lambda: proj_Q(0),
                        lambda: proj_V(0), lambda: proj_K(1)],
                    2: [lambda: proj_K(2), lambda: proj_Q(1),
                        lambda: proj_V(1), lambda: proj_Q(2)],
                    3: [lambda: proj_K(3), lambda: proj_Q(3),
                        lambda: proj_V(2), lambda: proj_V(3)],
                }

            def _attention(pools, Qp, Kp, Vsb, onescol, inject, carry):
                    (A_sb, A_db, psS, psAcc, psO, B_db, B_sm, Xf_db) = pools
                    s_only = (_PARTS_MODE == "s")

                    def emit_tail(o_ps, qc):
                        # softmax normalization + a2a scatter for chunk qc;
                        # deferred into the next chunk's exp shadow so the PE
                        # never stalls on the DVE reciprocal at a boundary.
                        for h in range(HPC):
                            rec = B_sm.tile([HD + 1, QCH], F32R, tag="rec")
                            with nc.allow_low_precision(
                                    reason="f32r is fp32-width; rounding only"):
                                if GEOM:
                                    nc.vector.reciprocal(rec[:],
                                                         o_ps[h][:])
                                else:
                                    nc.vector.reciprocal(rec[HD:HD + 1, :],
                                                         o_ps[h][HD:HD + 1, :])
                            # allocated from the S pool, NOT the accumulator
                            # pool: the acc tag must be free of attention-
                            # tail users so the other pipeline copy's
                            # projections can claim it at the start of this
                            # copy's attention
                            if GEOM:
                                rb_ps = psS.tile([128, QCH], F32,
                                                 name="rb_ps", tag="sps")
                                nc.tensor.matmul(rb_ps[:], shared["bpad"][:],
                                                 rec[:], start=True,
                                                 stop=True)
                            else:
                                rb_ps = psS.tile([HD, QCH], F32, tag="sps")
                                nc.tensor.matmul(rb_ps[:],
                                                 onescol[HD:HD + 1, :],
                                                 rec[HD:HD + 1, :],
                                                 start=True, stop=True,
                                                 tile_position=(HD, 0))
                            rb = B_sm.tile([HD, QCH], F32R, tag="rb_sb")
                            nc.vector.tensor_copy(rb[:], rb_ps[0:HD, :])
                            on = B_db.tile([HD, QCH], BF16, tag="on")
                            nc.vector.tensor_mul(on[:], o_ps[h][0:HD, :], rb[:])
                            # one strided DMA covers both destination cores;
                            # issued from the gpsimd queue so the SP queue stays
                            # a pure input-load queue and the next iteration's
                            # prefetch is not stuck behind it
                            nc.gpsimd.dma_start(
                                out=a2a_in[2 * qc:2 * qc + 2,
                                           h * HD:(h + 1) * HD, :]
                                .rearrange("r p n -> p r n"),
                                in_=on[:].rearrange("p (r n) -> p r n", r=2))

                    def emit_tail_pieces(o_ps, qc):
                        # same ops as emit_tail, split into three per-kt
                        # pieces so the PE stream never blocks on a fresh
                        # DVE result at a chunk boundary: recip first (DVE
                        # only), the rb broadcast matmuls a k-tile later
                        # (rec is ready by then), the payload mul + DMA last.
                        st = {}

                        def t1():
                            for h in range(HPC):
                                rec = B_sm.tile([HD + 1, QCH], F32R,
                                                name="rec", tag="rec")
                                with nc.allow_low_precision(
                                        reason="f32r is fp32-width"):
                                    if GEOM:
                                        nc.vector.reciprocal(rec[:],
                                                             o_ps[h][:])
                                    else:
                                        nc.vector.reciprocal(
                                            rec[HD:HD + 1, :],
                                            o_ps[h][HD:HD + 1, :])
                                st[h] = rec

                        def t2():
                            for h in range(HPC):
                                if GEOM:
                                    rb_ps = psS.tile([128, QCH], F32,
                                                     name="rb_ps", tag="sps")
                                    nc.tensor.matmul(rb_ps[:],
                                                     shared["bpad"][:],
                                                     st[h][:],
                                                     start=True, stop=True)
                                else:
                                    rb_ps = psS.tile([HD, QCH], F32,
                                                     name="rb_ps", tag="sps")
                                    nc.tensor.matmul(rb_ps[:],
                                                     onescol[HD:HD + 1, :],
                                                     st[h][HD:HD + 1, :],
                                                     start=True, stop=True,
                                                     tile_position=(HD, 0))
                                rb = B_sm.tile([HD, QCH], F32R,
                                               name="rb", tag="rb_sb")
                                nc.vector.tensor_copy(rb[:], rb_ps[0:HD, :])
                                st[(h, "rb")] = rb

                        def t3():
                            for h in range(HPC):
                                on = B_db.tile([HD, QCH], BF16,
                                               name="on", tag="on")
                                nc.vector.tensor_mul(on[:],
                                                     o_ps[h][0:HD, :],
                                                     st[(h, "rb")][:])
                                nc.gpsimd.dma_start(
                                    out=a2a_in[2 * qc:2 * qc + 2,
                                               h * HD:(h + 1) * HD, :]
                                    .rearrange("r p n -> p r n"),
                                    in_=on[:].rearrange("p (r n) -> p r n",
                                                        r=2))

                        return [t1, t2, t3]

                    pending = None
                    o_pending = None
                    extra = []
                    if carry:
                        o_pending, extra = carry
                    for qc in range(NQC):
                        sl = slice(qc * QCH, (qc + 1) * QCH)
                        # software-pipelined emission: S(kt+1) is emitted
                        # before O(kt) so the in-order PE fills the exp(kt)
                        # shadow with the next S pair instead of stalling.
                        def emit_s(kt):
                            s_ps = psS.tile([128, HPC, QCH], F32, tag="sps")
                            for h in range(HPC):
                                if SPLIT_S:
                                    for sh in range(2):
                                        ssl = slice(qc * QCH + sh * (QCH // 2),
                                                    qc * QCH + (sh + 1) * (QCH // 2))
                                        nc.tensor.matmul(
                                            s_ps[:, h, sh * (QCH // 2):(sh + 1) * (QCH // 2)],
                                            Kp[h * HD:(h + 1) * HD,
                                               kt * 128:(kt + 1) * 128],
                                            Qp[h * HD:(h + 1) * HD, ssl],
                                            start=True, stop=True,
                                            tile_position=(h * HD, 0))
                                elif GEOM:
                                    nc.tensor.matmul(
                                        s_ps[:, h, :],
                                        Kp[:, kt, h, :],
                                        Qp[:, sl],
                                        start=True, stop=True)
                                else:
                                    nc.tensor.matmul(
                                        s_ps[:, h, :],
                                        Kp[h * HD:(h + 1) * HD,
                                           kt * 128:(kt + 1) * 128],
                                        Qp[h * HD:(h + 1) * HD, sl],
                                        start=True, stop=True,
                                        tile_position=(h * HD, 0))
                            p_sb = B_db.tile([128, HPC, QCH], MDT, tag="p")
                            if DVE_W:
                                aw = QCH - DVE_W
                                nc.scalar.activation(
                                    out=p_sb[:, :, 0:aw], in_=s_ps[:, :, 0:aw],
                                    func=EXP, scale=256.0)
                                ue = B_sm.tile([128, HPC, DVE_W], F32, tag="ue")
                                with nc.allow_low_precision(
                                        reason="exp approx, ~1e-3 rel"):
                                    nc.vector._custom_dve(
                                        exp_op1, out=ue[:],
                                        in0=s_ps[:, :, aw:QCH],
                                        s0=EXP_C[0], s1=EXP_C[1], imm2=EXP_C[2])
                                    nc.vector._custom_dve(
                                        exp_op2,
                                        out=p_sb[:, :, aw:QCH],
                                        in0=ue[:])
                            elif EXP_SPLIT:
                                for h in range(HPC):
                                    nc.scalar.activation(
                                        out=p_sb[:, h, :], in_=s_ps[:, h, :],
                                        func=EXP, scale=256.0)
                            else:
                                nc.scalar.activation(out=p_sb[:], in_=s_ps[:],
                                                     func=EXP, scale=256.0)
                            return p_sb

                        def emit_o(kt, p_sb):
                            for h in range(HPC):
                                nc.tensor.matmul(
                                    o_ps[h][:], Vsb[:, kt, h, :], p_sb[:, h, :],
                                    start=(kt == 0), stop=(kt == NKT - 1))

                        p_prev = emit_s(0)
                        if o_pending is not None:
                            # previous chunk's last O, deferred past this
                            # chunk's first S so ACT's next exp is not stuck
                            # behind it in the in-order PE stream
                            o_pending()
                            o_pending = None
                        if pending is not None:
                            emit_tail(*pending)
                            pending = None
                        # the other pipeline copy's input copies and
                        # projection chunks ride inside this attention,
                        # paid out one piece per k-tile so the in-order PE
                        # stream never clumps them ahead of the next S tile
                        if INJ_FINE:
                            pend_inj = inject if inject else []
                        else:
                            pend_inj = (list(inject.get(qc, []))
                                        if inject else [])
                        if extra:
                            extra.pop(0)()
                        elif pend_inj:
                            pend_inj.pop(0)()
                        o_ps = None
                        if not s_only:
                            o_ps0 = psO.tile([HD + 1, QCH], F32, tag="oaug")
                            o_ps1 = psO.tile([HD + 1, QCH], F32, tag="oaug")
                            o_ps = [o_ps0, o_ps1]
                        for kt in range(1, NKT):
                            p_cur = emit_s(kt)
                            if INJ_EARLY:
                                if extra:
                                    extra.pop(0)()
                                elif pend_inj:
                                    pend_inj.pop(0)()
                            if not s_only:
                                emit_o(kt - 1, p_prev)
                            if not INJ_EARLY:
                                if extra:
                                    extra.pop(0)()
                                elif pend_inj:
                                    pend_inj.pop(0)()
                            p_prev = p_cur
                        if not s_only:
                            def _last_o(o=o_ps, p=p_prev):
                                for h in range(HPC):
                                    nc.tensor.matmul(
                                        o[h][:], Vsb[:, NKT - 1, h, :],
                                        p[:, h, :], start=False, stop=True)
                            o_pending = _last_o
                            if TAIL_SPREAD:
                                extra.extend(emit_tail_pieces(o_ps, qc))
                            else:
                                pending = (o_ps, qc)
                    if TAIL_SPREAD and inject is not None and carry is None:
                        # first attention of the loop body: hand the last
                        # chunk's deferred O + tail pieces to the second
                        # attention call's slots
                        return (o_pending, extra)
                    if o_pending is not None:
                        o_pending()
                    for t in extra:
                        t()
                    if pending is not None:
                        emit_tail(*pending)
                    return None

            def stage_c():
                with (
                    tc.tile_pool(name="stC", bufs=1) as C_sb,
                    tc.tile_pool(name="stC2", bufs=2) as C_db,
                    tc.tile_pool(name="psC", bufs=2, space="PSUM") as psC,
                ):
                    wpf = C_sb.tile([128, DCH, DIM], BF16, tag="wpf")
                    nc.sync.dma_start(
                        out=wpf[:], in_=wp_ext.rearrange("(s p) o -> p s o", p=128))
                    if BF16M:
                        wp = wpf
                    else:
                        wp = C_sb.tile([128, DCH, DIM], F32R, tag="wp")
                        nc.vector.tensor_copy(wp[:], wpf[:])
                    nc.gpsimd.collective_compute(
                        "AllToAll", mybir.AluOpType.bypass,
                        replica_groups=[list(range(NCORES))],
                        ins=[a2a_in[:]], outs=[a2a_out[:]])
                    gaf = C_sb.tile([CH, NCORES, 256], BF16, tag="gaf")
                    if BF16M:
                        ga = gaf
                        for r in range(NCORES):
                            nc.sync.dma_start(out=gaf[:, r, :], in_=a2a_out[r])
                    else:
                        ga = C_sb.tile([CH, NCORES, 256], F32R, tag="ga")
                        # per-src gather + widening copy (ga is a stationary
                        # operand): the first projection matmul starts after
                        # one chunk instead of the whole payload
                        for r in range(NCORES):
                            nc.sync.dma_start(out=gaf[:, r, :], in_=a2a_out[r])
                            nc.vector.tensor_copy(ga[:, r, :], gaf[:, r, :])
                    for nt in range(2):
                        for oc in range(2):
                            pp = psC.tile([128, 512], F32, tag="pp")
                            for src in range(NCORES):
                                nc.tensor.matmul(
                                    pp[:], ga[:, src, nt * 128:(nt + 1) * 128],
                                    wp[:, src, oc * 512:(oc + 1) * 512],
                                    start=(src == 0), stop=(src == NCORES - 1))
                            ob = C_db.tile([128, 512], F32, tag="ob")
                            nc.scalar.copy(ob[:], pp[:])
                            nc.sync.dma_start(
                                out=out_ext[nt * 128:(nt + 1) * 128,
                                            oc * 512:(oc + 1) * 512],
                                in_=ob[:])

            run(nrep)
            if with_c:
                stage_c()

    nc.compile()
    return nc


def _get_nc(nrep=1, n_cores=NCORES, with_c=True, parts="ab"):
    key = ("nc", nrep, n_cores, with_c, parts, SPLIT_S, XT_POOL, INJ_FINE, INJ_EARLY, TAIL_SPREAD, GEOM, BF16M, EXP_SPLIT, PB)
    if key not in _CACHE:
        _CACHE[key] = _build(nrep, n_cores, with_c, parts)
    return _CACHE[key]


def _prep_in_maps(x, wq, wk, wv, wp):
    bf16 = _np_bf16()
    x2 = np.asarray(x, np.float32).reshape(SEQ, DIM)
    xt = np.ascontiguousarray(x2.T).astype(bf16)
    wq = np.asarray(wq, np.float64)
    wk = np.asarray(wk, np.float64)
    wv = np.asarray(wv, np.float64)
    wp = np.asarray(wp, np.float32)
    cos2, s2 = _rope_tables()
    # fold 1/256 into wq so logits land pre-scaled for the (optional) DVE
    # exp path; the ACT exp undoes it for free via activation's scale arg
    scale = 1.0 / np.sqrt(HD) / 256.0
    wq = wq * scale
    ck = np.ascontiguousarray(cos2).astype(bf16)
    sk = np.ascontiguousarray(s2).astype(bf16)
    r2t = _r2t().astype(bf16)
    wpt = np.ascontiguousarray(wp.T).astype(bf16)
    maps = []
    for c in range(NCORES):
        ch = slice(c * CH, (c + 1) * CH)
        maps.append({
            "xt": xt,
            "wq_t": np.ascontiguousarray(wq[ch, :].T).astype(bf16),
            "wk_t": np.ascontiguousarray(wk[ch, :].T).astype(bf16),
            "wv_t": np.ascontiguousarray(wv[ch, :].T).astype(bf16),
            "wp_t": wpt,
            "cos_k": ck, "sin_k": sk,
            "r2t": r2t,
        })
    return maps


def kernel(x, wq, wk, wv, wp):
    from concourse.bass_utils import run_bass_kernel_spmd

    nc = _get_nc(1)
    maps = _prep_in_maps(x, wq, wk, wv, wp)
    res = run_bass_kernel_spmd(nc, maps, list(range(NCORES))).results
    out = np.concatenate([res[c]["out"] for c in range(NCORES)], axis=0)
    return out.reshape(1, SEQ, DIM).astype(np.float32)



# revision 22
# speedup vs baseline: 1.0120x; 1.0120x over previous
"""Trainium2 Bass kernel for multi-head attention with RoPE (nn_Attention).

Reference computation (B=1, N=2048, D=1024, 16 heads, hd=64):
    q = x @ wq.T; k = x @ wk.T; v = x @ wv.T      (reshaped to heads)
    q, k = rope(q), rope(k)
    out = softmax(q k^T / sqrt(hd)) v              (non-causal, full)
    return (out reshaped) @ wp.T

Sharding: tensor-parallel over heads — each of the 8 cores owns 2 heads for
QKV projection + SDPA, then an AllToAll redistributes the attention output
so each core computes the final projection for its 256 sequence rows with
the full wp.

Measured HW facts this kernel is shaped around (For_i slope probes):
  * the PE pays ~400ns whenever consecutive matmuls change tile geometry
    (contract/out partition config), so EVERY matmul here runs at the same
    (128,128) config: S stationaries are zero-padded per (k-tile, head) to
    128 contraction rows (the moving full-partition Qp read contributes 0
    from the other head's rows), and the softmax-normalization broadcast
    matmul is padded the same way (GEOM);
  * bf16 operand pairs run ~45ns/matmul faster than f32r (half-width
    LDWEIGHTS) and need no on-chip widening, so all matmul operands are
    bf16, straight from DRAM (BF16M); accumulation stays f32 in PSUM
    (rel err ~6e-3 vs the 2e-2 budget);
  * ACT exp costs ~(N+30)/1.2GHz with PSUM source — it is the pacing
    engine in steady state, so the PE stream is kept dense: deferred
    projection work is injected one ~1-ktile piece per k-tile slot
    (INJ_FINE) and each chunk's normalization tail is split into three
    pieces (reciprocal / broadcast / payload-mul+DMA) spread over the next
    chunk's slots and carried across the attention-call boundary
    (TAIL_SPREAD), so the in-order PE never blocks on a fresh DVE result.

The timing build software-pipelines iterations: the hardware loop body
holds TWO copies of the whole projection+attention stage with ping-pong
Q/K/V tiles and shared PSUM pools, so one copy's projections (PE/DVE/DMA)
overlap the other copy's softmax-bound attention.

Self-contained: only imports numpy + the concourse stack available in the
execution environment. kernel(**inputs) takes the full unsharded inputs and
returns the full output.
"""
import numpy as np

DIM = 1024
NHEADS = 16
HD = 64
SEQ = 2048
NCORES = 8
ROPE_BASE = 10000.0
HPC = NHEADS // NCORES      # heads per core = 2
CH = HPC * HD               # channels per core = 128
QCH = 512                   # q-chunk (free dim of S/P tiles)
NQC = SEQ // QCH            # 4
NKT = SEQ // 128            # 16 k-tiles
DCH = DIM // 128            # 8 contraction chunks

_CACHE = {}
_PARTS_MODE = "ab"
SPLIT_S = 0      # diag: split each S matmul into 2 halves
XT_POOL = 0     # widen xt on the Pool engine instead of DVE
INJ_FINE = 1    # flat, k-tile-granular injection of deferred projections
INJ_EARLY = 0   # emit injected piece before O(kt-1), widening the exp shadow
TAIL_SPREAD = 1  # spread emit_tail into per-kt pieces in the next chunk
GEOM = 1        # uniform (128,128) PE tile geometry: zero-padded S/rb stationaries
BF16M = 1       # bf16 matmul operands everywhere (halves LDWEIGHTS, no widening)
EXP_SPLIT = 0   # one exp per (kt, head): halves the exp->O wait granularity
PB = 0          # deeper SBUF pools (p_sb/qs/qct WAR slack; room freed by BF16M)

# Optional ACT/DVE softmax-exp split (0 = all exp on the scalar engine).
# Measured neutral at small widths — the two custom-DVE instructions cost
# ~650ns/k-tile fixed, which eats the ACT savings — so disabled.
DVE_W = 0
EXP_C = (1.0000677, 0.50037957, 0.1594775)


def _np_bf16():
    import concourse.mybir as mybir
    return mybir.dt.np(mybir.dt.bfloat16)


def _register_exp_ops():
    import re
    import concourse.dve_ops as dops
    from concourse.dve_spec import Spec, Src0, One, C0, C1, C2

    if "EXP_POLY_ANT" in dops.CUSTOM_DVE_SPECS:
        by = {o.name: o for o in dops.OPS}
        return by["EXP_POLY_ANT"], by["EXP_SQ8_ANT"]

    def np_poly(in0, in1, c0, c1, c2):
        f = np.float32
        t = np.asarray(in0, f)
        t2 = (t * t).astype(f)
        m1 = (t * np.asarray(c0, f)).astype(f)
        u1 = (m1 + f(1.0)).astype(f)
        m2 = (t * f(c2)).astype(f)
        u2 = (m2 + np.asarray(c1, f)).astype(f)
        return (u1 + (t2 * u2).astype(f)).astype(f)

    def np_sq8(in0, in1, c0, c1, c2):
        yv = np.asarray(in0, np.float32)
        for _ in range(8):
            yv = (yv * yv).astype(np.float32)
        return yv

    t2 = Src0 * Src0
    body1 = (Src0 * C0 + One) + t2 * (Src0 * C2 + C1)
    yb = Src0
    for _ in range(8):
        yb = yb * yb
    ops = []
    for name, body, ref in (("EXP_POLY_ANT", body1, np_poly),
                            ("EXP_SQ8_ANT", yb, np_sq8)):
        op = dops.DveOp(name, Spec(body=body, reference=ref), subdim=False,
                        uops_sha={})
        dops.OPS.append(op)
        dops.CUSTOM_DVE_SPECS[name] = op.spec
        dops._SUB_OPCODE_FOR_NAME[name] = (
            max(dops._SUB_OPCODE_FOR_NAME.values()) + 1)
        for ver in ("v3",):
            try:
                op.compile(ver)
            except ValueError as e:
                m = re.search(r'="([0-9a-f]+)"', str(e))
                assert m, f"no sha in error: {e}"
                op.uops_sha[ver] = m.group(1)
                op.compile(ver)
        ops.append(op)
    return ops


def _rope_tables():
    inv = 1.0 / (ROPE_BASE ** (np.arange(0, HD, 2, dtype=np.float64) / HD))
    t = np.arange(SEQ, dtype=np.float64)
    freqs = np.outer(t, inv)                      # [SEQ, 32]
    emb = np.concatenate([freqs, freqs], 1)       # [SEQ, 64]
    cosT = np.cos(emb).T                          # [64, SEQ]
    sinT = np.sin(emb).T
    sig = (np.arange(HD) + 32) % HD
    sT = sinT[sig]                                # shifted sin
    cos2 = np.concatenate([cosT, cosT], 0)        # [128, SEQ] (2 heads)
    s2 = np.concatenate([sT, sT], 0)
    return cos2, s2


def _r2t():
    # rotate-half matrix R (per head), block-diagonal over the 2 heads; we
    # pass R2.T as the stationary matmul operand.
    R = np.zeros((HD, HD), np.float64)
    for j in range(32):
        R[j, j + 32] = -1.0
        R[j + 32, j] = 1.0
    R2 = np.zeros((CH, CH), np.float64)
    R2[0:HD, 0:HD] = R
    R2[HD:CH, HD:CH] = R
    return np.ascontiguousarray(R2.T)


def _build(nrep=1, n_cores=NCORES, with_c=True, parts="ab"):
    global _PARTS_MODE
    _PARTS_MODE = parts
    import concourse.mybir as mybir
    import concourse.tile as tile
    from concourse import bacc
    from concourse.masks import make_identity

    F32 = mybir.dt.float32
    F32R = mybir.dt.float32r
    BF16 = mybir.dt.bfloat16
    MDT = BF16 if BF16M else F32R
    EXP = mybir.ActivationFunctionType.Exp

    if DVE_W:
        exp_op1, exp_op2 = _register_exp_ops()

    nc = bacc.Bacc("TRN2", target_bir_lowering=False, debug=False,
                   num_devices=n_cores)

    xt_ext = nc.dram_tensor("xt", [DIM, SEQ], BF16, kind="ExternalInput")
    wq_ext = nc.dram_tensor("wq_t", [DIM, CH], BF16, kind="ExternalInput")
    wk_ext = nc.dram_tensor("wk_t", [DIM, CH], BF16, kind="ExternalInput")
    wv_ext = nc.dram_tensor("wv_t", [DIM, CH], BF16, kind="ExternalInput")
    wp_ext = nc.dram_tensor("wp_t", [DIM, DIM], BF16, kind="ExternalInput")
    ck_ext = nc.dram_tensor("cos_k", [CH, SEQ], BF16, kind="ExternalInput")
    sk_ext = nc.dram_tensor("sin_k", [CH, SEQ], BF16, kind="ExternalInput")
    r2t_ext = nc.dram_tensor("r2t", [CH, CH], BF16, kind="ExternalInput")
    out_ext = nc.dram_tensor("out", [SEQ // NCORES, DIM], F32,
                             kind="ExternalOutput")
    a2a_in = nc.dram_tensor("a2a_in", [NCORES, CH, SEQ // NCORES], BF16)
    a2a_out = nc.dram_tensor("a2a_out", [NCORES, CH, SEQ // NCORES], BF16)

    with tile.TileContext(nc) as tc:
        with tc.tile_pool(name="persist", bufs=1) as P1:

            def make_set(sfx):
                Qp = P1.tile([CH, SEQ], MDT, tag="Qp" + sfx)
                if GEOM:
                    # per-(kt, head) zero-padded stationaries: the head's 64
                    # rows live at their natural partitions, the other 64
                    # rows stay zero so the full-partition moving Qp read
                    # contributes nothing from the other head. Keeps every
                    # matmul at the same (128,128) PE tile config - geometry
                    # switches cost ~400ns each on HW.
                    Kp = P1.tile([128, NKT, HPC, 128], MDT, tag="Kp" + sfx)
                    if BF16M:
                        nc.vector.memset(Kp[:], 0.0)
                    else:
                        nc.vector.memset(Kp[:].bitcast(F32), 0.0)
                else:
                    Kp = P1.tile([CH, SEQ], MDT, tag="Kp" + sfx)
                Vsb = P1.tile([128, NKT, HPC, HD + 1], MDT, tag="Vsb" + sfx)
                if BF16M and GEOM:
                    nc.vector.memset(Vsb[:, :, :, HD], 1.0)
                onescol = P1.tile([128, HD], F32R, tag="ones" + sfx)
                return (Qp, Kp, Vsb, onescol)

            shared = {}

            def run(nrep_eff):
                ident0 = P1.tile([128, 128], F32, name="ident0", tag="ident0")
                identr0 = P1.tile([128, 128], MDT, name="identr0",
                                  tag="identr0")
                make_identity(nc, ident0[:])
                nc.vector.tensor_copy(identr0[:], ident0[:])
                shared["identr"] = identr0
                if GEOM:
                    bpad = P1.tile([HD + 1, 128], F32R, name="bpad",
                                   tag="bpad")
                    nc.vector.memset(bpad[:].bitcast(F32), 0.0)
                    nc.vector.memset(bpad[HD:HD + 1, :].bitcast(F32), 1.0)
                    shared["bpad"] = bpad
                with (
                    tc.tile_pool(name="stA", bufs=1) as A_sb,
                    tc.tile_pool(name="stA2", bufs=3 if PB else 2) as A_db,
                    tc.tile_pool(name="psS", bufs=2, space="PSUM") as psS,
                    tc.tile_pool(name="psAcc", bufs=2, space="PSUM") as psAcc,
                    tc.tile_pool(name="psO", bufs=2, space="PSUM") as psO,
                    tc.tile_pool(name="stB", bufs=5 if PB else 3) as B_db,
                    tc.tile_pool(name="stBs", bufs=4 if PB else 3) as B_sm,
                    tc.tile_pool(name="stXf", bufs=3 if GEOM else 4) as Xf_db,
                ):
                    pools = (A_sb, A_db, psS, psAcc, psO, B_db, B_sm, Xf_db)
                    setA = make_set("A")
                    if nrep_eff == 1:
                        stage_proj(pools, *setA)
                        _attention(pools, *setA, None, None)
                    else:
                        assert nrep_eff % 2 == 0
                        setB = make_set("B")
                        # prologue: project set A once; each loop body then
                        # re-projects the OTHER set inside this set's
                        # attention (chunk-granular injection keeps the PE
                        # stream dense while ACT paces the softmax)
                        stage_proj(pools, *setA)
                        with tc.For_i(0, nrep_eff // 2, 1) as _i:
                            injB = stage_proj(pools, *setB, defer=True)
                            carry = _attention(pools, *setA, injB, None)
                            injA = stage_proj(pools, *setA, defer=True)
                            _attention(pools, *setB, injA, carry)

            def stage_proj(pools, Qp, Kp, Vsb, onescol, defer=False):
                (A_sb, A_db, psS, psAcc, psO, B_db, B_sm, Xf_db) = pools

                def init_ones():
                    if BF16M and GEOM:
                        return
                    aux1 = A_sb.tile([128, HD], F32, tag="aux1")
                    nc.vector.memset(aux1[:], 1.0)
                    nc.vector.tensor_copy(onescol[:], aux1[:])

                    nc.vector.tensor_copy(
                        Vsb[:, :, :, HD],
                        aux1[:, 0:NKT * HPC].rearrange("p (k h) -> p k h",
                                                       h=HPC))

                # ---- inputs: bf16 from DRAM; widened to f32r on chip.
                # All DMAs are emitted immediately (the SP queue prefetches
                # as staging slots free up); the widening copies and the
                # projections are returned as per-qc closures when defer=True
                # so they ride inside the other pipeline copy's attention.
                wq = A_sb.tile([128, DCH, CH], MDT, tag="wq")
                wk = A_sb.tile([128, DCH, CH], MDT, tag="wk")
                wv = A_sb.tile([128, DCH, CH], MDT, tag="wv")
                xt = A_sb.tile([128, DCH, SEQ], MDT, tag="xt")
                xt_r = xt_ext.rearrange("(c p) n -> p c n", p=128)
                ck = A_sb.tile([CH, SEQ], BF16, tag="ck")
                sk = A_sb.tile([CH, SEQ], BF16, tag="sk")
                if BF16M:
                    # bf16 operands feed the PE directly from DRAM: no
                    # staging, no widening copies
                    r2t = A_db.tile([CH, CH], BF16, tag="r2tf")
                    nc.sync.dma_start(
                        out=wk[:],
                        in_=wk_ext.rearrange("(c p) j -> p c j", p=128))
                    nc.sync.dma_start(out=r2t[:], in_=r2t_ext[:])
                    nc.sync.dma_start(
                        out=wv[:],
                        in_=wv_ext.rearrange("(c p) j -> p c j", p=128))
                    for d in range(DCH):
                        nc.sync.dma_start(out=xt[:, d, :], in_=xt_r[:, d, :])
                    nc.sync.dma_start(out=sk[:], in_=sk_ext[:])
                    nc.sync.dma_start(out=ck[:], in_=ck_ext[:])
                    nc.sync.dma_start(
                        out=wq[:],
                        in_=wq_ext.rearrange("(c p) j -> p c j", p=128))
                else:
                    wkf = A_db.tile([128, DCH, CH], BF16, tag="wf")
                    nc.sync.dma_start(
                        out=wkf[:],
                        in_=wk_ext.rearrange("(c p) j -> p c j", p=128))
                    r2t = A_sb.tile([CH, CH], F32R, tag="r2t")
                    r2tf = A_db.tile([CH, CH], BF16, tag="r2tf")
                    nc.sync.dma_start(out=r2tf[:], in_=r2t_ext[:])
                    wvf = A_db.tile([128, DCH, CH], BF16, tag="wf")
                    nc.sync.dma_start(
                        out=wvf[:],
                        in_=wv_ext.rearrange("(c p) j -> p c j", p=128))
                    xtfs = []
                    for d in range(DCH):
                        xtf = Xf_db.tile([128, SEQ], BF16, tag="xtf")
                        nc.sync.dma_start(out=xtf[:], in_=xt_r[:, d, :])
                        xtfs.append(xtf)
                    nc.sync.dma_start(out=sk[:], in_=sk_ext[:])
                    nc.sync.dma_start(out=ck[:], in_=ck_ext[:])
                    wqf = A_db.tile([128, DCH, CH], BF16, tag="wf")
                    nc.sync.dma_start(
                        out=wqf[:],
                        in_=wq_ext.rearrange("(c p) j -> p c j", p=128))

                identr = shared["identr"]

                def copies():
                    if not BF16M:
                        # weights widen on the idle gpsimd; xt on the DVE
                        # where the bf16->f32r copy runs in 2x mode
                        nc.gpsimd.tensor_copy(wk[:], wkf[:])
                        nc.vector.tensor_copy(r2t[:], r2tf[:])
                        nc.gpsimd.tensor_copy(wv[:], wvf[:])
                        xteng = nc.gpsimd if XT_POOL else nc.vector
                        for d in range(DCH):
                            xteng.tensor_copy(xt[:, d, :], xtfs[d])
                        nc.gpsimd.tensor_copy(wq[:], wqf[:])

                def rope_add(dst, qct, acc, c, is_k):
                    slc = slice(c * QCH, (c + 1) * QCH)
                    if not (GEOM and is_k):
                        nc.vector.tensor_add(dst[:, slc], qct[:], acc[:])
                        return
                    kts = slice(c * (QCH // 128), (c + 1) * (QCH // 128))
                    for h in range(HPC):
                        hp = slice(h * HD, (h + 1) * HD)
                        nc.vector.tensor_add(
                            dst[hp, kts, h, :],
                            qct[hp, :].rearrange("p (k n) -> p k n", n=128),
                            acc[hp, :].rearrange("p (k n) -> p k n", n=128))

                def proj_rope(w_sb, dst, c, is_k=False):
                    # shared K/Q chunk: project into a 1-bank accumulator,
                    # RoPE via two DVE muls + in-place rot matmul + add
                    slc = slice(c * QCH, (c + 1) * QCH)
                    acc = psAcc.tile([CH, QCH], F32, tag="acc")
                    for d in range(DCH):
                        nc.tensor.matmul(acc[:], w_sb[:, d, :], xt[:, d, slc],
                                         start=(d == 0), stop=(d == DCH - 1))
                    qs = A_db.tile([CH, QCH], MDT, tag="qs")
                    nc.vector.tensor_mul(qs[:], acc[:], sk[:, slc])
                    qct = A_db.tile([CH, QCH], MDT, tag="qct")
                    nc.vector.tensor_mul(qct[:], acc[:], ck[:, slc])
                    nc.tensor.matmul(acc[:], r2t[:], qs[:],
                                     start=True, stop=True)
                    rope_add(dst, qct, acc, c, is_k)

                def proj_K(c):
                    proj_rope(wk, Kp, c, is_k=True)

                def proj_Q(c):
                    proj_rope(wq, Qp, c)

                def proj_V(c):
                    # project, copy out of psum, PE-transpose into Vsb
                    slc = slice(c * QCH, (c + 1) * QCH)
                    vacc = psAcc.tile([CH, QCH], F32, tag="acc")
                    for d in range(DCH):
                        nc.tensor.matmul(vacc[:], wv[:, d, :], xt[:, d, slc],
                                         start=(d == 0), stop=(d == DCH - 1))
                    vt = A_db.tile([CH, QCH], MDT, tag="vt")
                    nc.vector.tensor_copy(vt[:], vacc[:])
                    for b in range(QCH // 128):
                        kti = c * (QCH // 128) + b
                        ps_t = psAcc.tile([128, 128], MDT, tag="acc")
                        nc.tensor.transpose(
                            ps_t[:], vt[:, b * 128:(b + 1) * 128], identr[:])
                        nc.vector.tensor_copy(
                            Vsb[:, kti, :, 0:HD],
                            ps_t[:].rearrange("p (h j) -> p h j", h=HPC))

                def proj_rope_pieces(w_sb, dst, c, is_k=False):
                    # same work as proj_rope, split into ~1-ktile PE pieces
                    slc = slice(c * QCH, (c + 1) * QCH)
                    st = {}

                    def p1():
                        st["acc"] = psAcc.tile([CH, QCH], F32, name="acc",
                                               tag="acc")
                        for d in range(DCH // 2):
                            nc.tensor.matmul(st["acc"][:], w_sb[:, d, :],
                                             xt[:, d, slc],
                                             start=(d == 0), stop=False)

                    def p2():
                        for d in range(DCH // 2, DCH):
                            nc.tensor.matmul(st["acc"][:], w_sb[:, d, :],
                                             xt[:, d, slc],
                                             start=False, stop=(d == DCH - 1))

                    def p3():
                        st["qs"] = A_db.tile([CH, QCH], MDT, name="qs",
                                             tag="qs")
                        nc.vector.tensor_mul(st["qs"][:], st["acc"][:],
                                             sk[:, slc])
                        st["qct"] = A_db.tile([CH, QCH], MDT, name="qct",
                                              tag="qct")
                        nc.vector.tensor_mul(st["qct"][:], st["acc"][:],
                                             ck[:, slc])
                        nc.tensor.matmul(st["acc"][:], r2t[:], st["qs"][:],
                                         start=True, stop=True)

                    def p4():
                        rope_add(dst, st["qct"], st["acc"], c, is_k)

                    return [p1, p2, p3, p4]

                def proj_V_pieces(c):
                    slc = slice(c * QCH, (c + 1) * QCH)
                    st = {}

                    def p1():
                        st["vacc"] = psAcc.tile([CH, QCH], F32, name="vacc",
                                                tag="acc")
                        for d in range(DCH // 2):
                            nc.tensor.matmul(st["vacc"][:], wv[:, d, :],
                                             xt[:, d, slc],
                                             start=(d == 0), stop=False)

                    def p2():
                        for d in range(DCH // 2, DCH):
                            nc.tensor.matmul(st["vacc"][:], wv[:, d, :],
                                             xt[:, d, slc],
                                             start=False, stop=(d == DCH - 1))

                    def trans(b):
                        kti = c * (QCH // 128) + b
                        ps_t = psAcc.tile([128, 128], MDT, name="ps_t",
                                          tag="acc")
                        nc.tensor.transpose(
                            ps_t[:], st["vt"][:, b * 128:(b + 1) * 128],
                            identr[:])
                        nc.vector.tensor_copy(
                            Vsb[:, kti, :, 0:HD],
                            ps_t[:].rearrange("p (h j) -> p h j", h=HPC))

                    def p3():
                        st["vt"] = A_db.tile([CH, QCH], MDT, name="vt",
                                             tag="vt")
                        nc.vector.tensor_copy(st["vt"][:], st["vacc"][:])
                        trans(0)
                        trans(1)

                    def p4():
                        trans(2)
                        trans(3)

                    return [p1, p2, p3, p4]

                if not defer:
                    copies()
                    init_ones()
                    for c in range(NQC):
                        proj_K(c)
                    for c in range(NQC):
                        proj_Q(c)
                    for c in range(NQC):
                        proj_V(c)
                    return None
                if INJ_FINE:
                    pieces = [copies, init_ones]
                    for c in range(NQC):
                        pieces += proj_rope_pieces(wk, Kp, c, is_k=True)
                    for c in range(NQC):
                        pieces += proj_rope_pieces(wq, Qp, c)
                    for c in range(NQC):
                        pieces += proj_V_pieces(c)
                    return pieces
                # four projection chunks per qc slot (~7.8us PE each)
                # so no attention chunk's PE slack is oversubscribed
                return {
                    0: [copies, init_ones],
                    1: [lambda: proj_K(0), lambda: proj_Q(0),
                        lambda: proj_V(0), lambda: proj_K(1)],
                    2: [lambda: proj_K(2), lambda: proj_Q(1),
                        lambda: proj_V(1), lambda: proj_Q(2)],
                    3: [lambda: proj_K(3), lambda: proj_Q(3),
                        lambda: proj_V(2), lambda: proj_V(3)],
                }

            def _attention(pools, Qp, Kp, Vsb, onescol, inject, carry):
                    (A_sb, A_db, psS, psAcc, psO, B_db, B_sm, Xf_db) = pools
                    s_only = (_PARTS_MODE == "s")

                    def emit_tail(o_ps, qc):
                        # softmax normalization + a2a scatter for chunk qc;
                        # deferred into the next chunk's exp shadow so the PE
                        # never stalls on the DVE reciprocal at a boundary.
                        for h in range(HPC):
                            rec = B_sm.tile([HD + 1, QCH], F32R, tag="rec")
                            with nc.allow_low_precision(
                                    reason="f32r is fp32-width; rounding only"):
                                if GEOM:
                                    nc.vector.reciprocal(rec[:],
                                                         o_ps[h][:])
                                else:
                                    nc.vector.reciprocal(rec[HD:HD + 1, :],
                                                         o_ps[h][HD:HD + 1, :])
                            # allocated from the S pool, NOT the accumulator
                            # pool: the acc tag must be free of attention-
                            # tail users so the other pipeline copy's
                            # projections can claim it at the start of this
                            # copy's attention
                            if GEOM:
                                rb_ps = psS.tile([128, QCH], F32,
                                                 name="rb_ps", tag="sps")
                                nc.tensor.matmul(rb_ps[:], shared["bpad"][:],
                                                 rec[:], start=True,
                                                 stop=True)
                            else:
                                rb_ps = psS.tile([HD, QCH], F32, tag="sps")
                                nc.tensor.matmul(rb_ps[:],
                                                 onescol[HD:HD + 1, :],
                                                 rec[HD:HD + 1, :],
                                                 start=True, stop=True,
                                                 tile_position=(HD, 0))
                            rb = B_sm.tile([HD, QCH], F32R, tag="rb_sb")
                            nc.vector.tensor_copy(rb[:], rb_ps[0:HD, :])
                            on = B_db.tile([HD, QCH], BF16, tag="on")
                            nc.vector.tensor_mul(on[:], o_ps[h][0:HD, :], rb[:])
                            # one strided DMA covers both destination cores;
                            # issued from the gpsimd queue so the SP queue stays
                            # a pure input-load queue and the next iteration's
                            # prefetch is not stuck behind it
                            nc.gpsimd.dma_start(
                                out=a2a_in[2 * qc:2 * qc + 2,
                                           h * HD:(h + 1) * HD, :]
                                .rearrange("r p n -> p r n"),
                                in_=on[:].rearrange("p (r n) -> p r n", r=2))

                    def emit_tail_pieces(o_ps, qc):
                        # same ops as emit_tail, split into three per-kt
                        # pieces so the PE stream never blocks on a fresh
                        # DVE result at a chunk boundary: recip first (DVE
                        # only), the rb broadcast matmuls a k-tile later
                        # (rec is ready by then), the payload mul + DMA last.
                        st = {}

                        def t1():
                            for h in range(HPC):
                                rec = B_sm.tile([HD + 1, QCH], F32R,
                                                name="rec", tag="rec")
                                with nc.allow_low_precision(
                                        reason="f32r is fp32-width"):
                                    if GEOM:
                                        nc.vector.reciprocal(rec[:],
                                                             o_ps[h][:])
                                    else:
                                        nc.vector.reciprocal(
                                            rec[HD:HD + 1, :],
                                            o_ps[h][HD:HD + 1, :])
                                st[h] = rec

                        def t2():
                            for h in range(HPC):
                                if GEOM:
                                    rb_ps = psS.tile([128, QCH], F32,
                                                     name="rb_ps", tag="sps")
                                    nc.tensor.matmul(rb_ps[:],
                                                     shared["bpad"][:],
                                                     st[h][:],
                                                     start=True, stop=True)
                                else:
                                    rb_ps = psS.tile([HD, QCH], F32,
                                                     name="rb_ps", tag="sps")
                                    nc.tensor.matmul(rb_ps[:],
                                                     onescol[HD:HD + 1, :],
                                                     st[h][HD:HD + 1, :],
                                                     start=True, stop=True,
                                                     tile_position=(HD, 0))
                                rb = B_sm.tile([HD, QCH], F32R,
                                               name="rb", tag="rb_sb")
                                nc.vector.tensor_copy(rb[:], rb_ps[0:HD, :])
                                st[(h, "rb")] = rb

                        def t3():
                            for h in range(HPC):
                                on = B_db.tile([HD, QCH], BF16,
                                               name="on", tag="on")
                                nc.vector.tensor_mul(on[:],
                                                     o_ps[h][0:HD, :],
                                                     st[(h, "rb")][:])
                                nc.gpsimd.dma_start(
                                    out=a2a_in[2 * qc:2 * qc + 2,
                                               h * HD:(h + 1) * HD, :]
                                    .rearrange("r p n -> p r n"),
                                    in_=on[:].rearrange("p (r n) -> p r n",
                                                        r=2))

                        return [t1, t2, t3]

                    pending = None
                    o_pending = None
                    extra = []
                    if carry:
                        o_pending, extra = carry
                    for qc in range(NQC):
                        sl = slice(qc * QCH, (qc + 1) * QCH)
                        # software-pipelined emission: S(kt+1) is emitted
                        # before O(kt) so the in-order PE fills the exp(kt)
                        # shadow with the next S pair instead of stalling.
                        def emit_s(kt):
                            s_ps = psS.tile([128, HPC, QCH], F32, tag="sps")
                            for h in range(HPC):
                                if SPLIT_S:
                                    for sh in range(2):
                                        ssl = slice(qc * QCH + sh * (QCH // 2),
                                                    qc * QCH + (sh + 1) * (QCH // 2))
                                        nc.tensor.matmul(
                                            s_ps[:, h, sh * (QCH // 2):(sh + 1) * (QCH // 2)],
                                            Kp[h * HD:(h + 1) * HD,
                                               kt * 128:(kt + 1) * 128],
                                            Qp[h * HD:(h + 1) * HD, ssl],
                                            start=True, stop=True,
                                            tile_position=(h * HD, 0))
                                elif GEOM:
                                    nc.tensor.matmul(
                                        s_ps[:, h, :],
                                        Kp[:, kt, h, :],
                                        Qp[:, sl],
                                        start=True, stop=True)
                                else:
                                    nc.tensor.matmul(
                                        s_ps[:, h, :],
                                        Kp[h * HD:(h + 1) * HD,
                                           kt * 128:(kt + 1) * 128],
                                        Qp[h * HD:(h + 1) * HD, sl],
                                        start=True, stop=True,
                                        tile_position=(h * HD, 0))
                            p_sb = B_db.tile([128, HPC, QCH], MDT, tag="p")
                            if DVE_W:
                                aw = QCH - DVE_W
                                nc.scalar.activation(
                                    out=p_sb[:, :, 0:aw], in_=s_ps[:, :, 0:aw],
                                    func=EXP, scale=256.0)
                                ue = B_sm.tile([128, HPC, DVE_W], F32, tag="ue")
                                with nc.allow_low_precision(
                                        reason="exp approx, ~1e-3 rel"):
                                    nc.vector._custom_dve(
                                        exp_op1, out=ue[:],
                                        in0=s_ps[:, :, aw:QCH],
                                        s0=EXP_C[0], s1=EXP_C[1], imm2=EXP_C[2])
                                    nc.vector._custom_dve(
                                        exp_op2,
                                        out=p_sb[:, :, aw:QCH],
                                        in0=ue[:])
                            elif EXP_SPLIT:
                                for h in range(HPC):
                                    nc.scalar.activation(
                                        out=p_sb[:, h, :], in_=s_ps[:, h, :],
                                        func=EXP, scale=256.0)
                            else:
                                nc.scalar.activation(out=p_sb[:], in_=s_ps[:],
                                                     func=EXP, scale=256.0)
                            return p_sb

                        def emit_o(kt, p_sb):
                            for h in range(HPC):
                                nc.tensor.matmul(
                                    o_ps[h][:], Vsb[:, kt, h, :], p_sb[:, h, :],
                                    start=(kt == 0), stop=(kt == NKT - 1))

                        p_prev = emit_s(0)
                        if o_pending is not None:
                            # previous chunk's last O, deferred past this
                            # chunk's first S so ACT's next exp is not stuck
                            # behind it in the in-order PE stream
                            o_pending()
                            o_pending = None
                        if pending is not None:
                            emit_tail(*pending)
                            pending = None
                        # the other pipeline copy's input copies and
                        # projection chunks ride inside this attention,
                        # paid out one piece per k-tile so the in-order PE
                        # stream never clumps them ahead of the next S tile
                        if INJ_FINE:
                            pend_inj = inject if inject else []
                        else:
                            pend_inj = (list(inject.get(qc, []))
                                        if inject else [])
                        if extra:
                            extra.pop(0)()
                        elif pend_inj:
                            pend_inj.pop(0)()
                        o_ps = None
                        if not s_only:
                            o_ps0 = psO.tile([HD + 1, QCH], F32, tag="oaug")
                            o_ps1 = psO.tile([HD + 1, QCH], F32, tag="oaug")
                            o_ps = [o_ps0, o_ps1]
                        for kt in range(1, NKT):
                            p_cur = emit_s(kt)
                            if INJ_EARLY:
                                if extra:
                                    extra.pop(0)()
                                elif pend_inj:
                                    pend_inj.pop(0)()
                            if not s_only:
                                emit_o(kt - 1, p_prev)
                            if not INJ_EARLY:
                                if extra:
                                    extra.pop(0)()
                                elif pend_inj:
                                    pend_inj.pop(0)()
                            p_prev = p_cur
                        if not s_only:
                            def _last_o(o=o_ps, p=p_prev):
                                for h in range(HPC):
                                    nc.tensor.matmul(
                                        o[h][:], Vsb[:, NKT - 1, h, :],
                                        p[:, h, :], start=False, stop=True)
                            o_pending = _last_o
                            if TAIL_SPREAD:
                                extra.extend(emit_tail_pieces(o_ps, qc))
                            else:
                                pending = (o_ps, qc)
                    if TAIL_SPREAD and inject is not None and carry is None:
                        # first attention of the loop body: hand the last
                        # chunk's deferred O + tail pieces to the second
                        # attention call's slots
                        return (o_pending, extra)
                    if o_pending is not None:
                        o_pending()
                    for t in extra:
                        t()
                    if pending is not None:
                        emit_tail(*pending)
                    return None

            def stage_c():
                with (
                    tc.tile_pool(name="stC", bufs=1) as C_sb,
                    tc.tile_pool(name="stC2", bufs=2) as C_db,
                    tc.tile_pool(name="psC", bufs=2, space="PSUM") as psC,
                ):
                    wpf = C_sb.tile([128, DCH, DIM], BF16, tag="wpf")
                    nc.sync.dma_start(
                        out=wpf[:], in_=wp_ext.rearrange("(s p) o -> p s o", p=128))
                    if BF16M:
                        wp = wpf
                    else:
                        wp = C_sb.tile([128, DCH, DIM], F32R, tag="wp")
                        nc.vector.tensor_copy(wp[:], wpf[:])
                    nc.gpsimd.collective_compute(
                        "AllToAll", mybir.AluOpType.bypass,
                        replica_groups=[list(range(NCORES))],
                        ins=[a2a_in[:]], outs=[a2a_out[:]])
                    gaf = C_sb.tile([CH, NCORES, 256], BF16, tag="gaf")
                    if BF16M:
                        ga = gaf
                        for r in range(NCORES):
                            nc.sync.dma_start(out=gaf[:, r, :], in_=a2a_out[r])
                    else:
                        ga = C_sb.tile([CH, NCORES, 256], F32R, tag="ga")
                        # per-src gather + widening copy (ga is a stationary
                        # operand): the first projection matmul starts after
                        # one chunk instead of the whole payload
                        for r in range(NCORES):
                            nc.sync.dma_start(out=gaf[:, r, :], in_=a2a_out[r])
                            nc.vector.tensor_copy(ga[:, r, :], gaf[:, r, :])
                    for nt in range(2):
                        for oc in range(2):
                            pp = psC.tile([128, 512], F32, tag="pp")
                            for src in range(NCORES):
                                nc.tensor.matmul(
                                    pp[:], ga[:, src, nt * 128:(nt + 1) * 128],
                                    wp[:, src, oc * 512:(oc + 1) * 512],
                                    start=(src == 0), stop=(src == NCORES - 1))
                            ob = C_db.tile([128, 512], F32, tag="ob")
                            nc.scalar.copy(ob[:], pp[:])
                            nc.sync.dma_start(
                                out=out_ext[nt * 128:(nt + 1) * 128,
                                            oc * 512:(oc + 1) * 512],
                                in_=ob[:])

            run(nrep)
            if with_c:
                stage_c()

    nc.compile()
    return nc


def _get_nc(nrep=1, n_cores=NCORES, with_c=True, parts="ab"):
    key = ("nc", nrep, n_cores, with_c, parts, SPLIT_S, XT_POOL, INJ_FINE, INJ_EARLY, TAIL_SPREAD, GEOM, BF16M, EXP_SPLIT, PB)
    if key not in _CACHE:
        _CACHE[key] = _build(nrep, n_cores, with_c, parts)
    return _CACHE[key]


def _prep_in_maps(x, wq, wk, wv, wp):
    bf16 = _np_bf16()
    x2 = np.asarray(x, np.float32).reshape(SEQ, DIM)
    xt = np.ascontiguousarray(x2.T).astype(bf16)
    wq = np.asarray(wq, np.float64)
    wk = np.asarray(wk, np.float64)
    wv = np.asarray(wv, np.float64)
    wp = np.asarray(wp, np.float32)
    cos2, s2 = _rope_tables()
    # fold 1/256 into wq so logits land pre-scaled for the (optional) DVE
    # exp path; the ACT exp undoes it for free via activation's scale arg
    scale = 1.0 / np.sqrt(HD) / 256.0
    wq = wq * scale
    ck = np.ascontiguousarray(cos2).astype(bf16)
    sk = np.ascontiguousarray(s2).astype(bf16)
    r2t = _r2t().astype(bf16)
    wpt = np.ascontiguousarray(wp.T).astype(bf16)
    maps = []
    for c in range(NCORES):
        ch = slice(c * CH, (c + 1) * CH)
        maps.append({
            "xt": xt,
            "wq_t": np.ascontiguousarray(wq[ch, :].T).astype(bf16),
            "wk_t": np.ascontiguousarray(wk[ch, :].T).astype(bf16),
            "wv_t": np.ascontiguousarray(wv[ch, :].T).astype(bf16),
            "wp_t": wpt,
            "cos_k": ck, "sin_k": sk,
            "r2t": r2t,
        })
    return maps


def kernel(x, wq, wk, wv, wp):
    from concourse.bass_utils import run_bass_kernel_spmd

    nc = _get_nc(1)
    maps = _prep_in_maps(x, wq, wk, wv, wp)
    res = run_bass_kernel_spmd(nc, maps, list(range(NCORES))).results
    out = np.concatenate([res[c]["out"] for c in range(NCORES)], axis=0)
    return out.reshape(1, SEQ, DIM).astype(np.float32)

